# revision 27
# baseline (speedup 1.0000x reference)
"""Trainium2 Bass kernel for AdvancedWeightPredictorNetwork (retrieval_knn).

Strategy (8 NeuronCores, data-parallel over rows of x):
  - Each core owns a 1024-row shard of x [8192, 256]; x^T is replicated
    (pre-laid-out in fp16 by the host as part of sharding).
  - Distance ranking: G' = x_shard @ x_all^T - r2_all/2 via fp16 matmuls
    into PSUM (k-chunks: two 128-feature halves + one row carrying
    -|x_j|^2/2, so ranking by G' equals ranking by -cdist^2).
    Per 2048-col PSUM chunk, vector.max (top-8, sorted desc) reduces
    the chunk straight from PSUM; a merge max over 4x8 candidates gives
    the global top-8 per row. Entry 0 is always the row itself; entries
    1..5 are the 5 nearest: knn = sqrt(r2_i - 2*s) with exact fp32
    r2_i (= sum x^2 from the local-stats pass). The tie-break noise
    matrix only permutes neighbors whose distances agree to ~1e-6 (far
    below tolerance), so it is not streamed.
  - Phase A is a pure dense matmul stream (PE clock-gate stays warm);
    phase B (cluster softmax / stats / MLP) is batched per op across
    row tiles to minimize ACT table reloads and DVE op count. exp() is
    applied without max-subtraction: inputs are bounded (|x| < 6,
    logits in [-25, 0]) so fp32 exp is safe and matches softmax math.
  - Scalar losses: per-core intra partial + inter computed on device;
    host sums the 8 partials.
"""

import numpy as np

import concourse.bacc as bacc
import concourse.tile as tile
import concourse.mybir as mybir
from concourse.bass_utils import run_bass_kernel_spmd

F32 = mybir.dt.float32
F16 = mybir.dt.float16
ALU = mybir.AluOpType
ACTF = mybir.ActivationFunctionType
AX = mybir.AxisListType

B = 8192        # total rows
NCORES = 8
S = B // NCORES  # rows per core (1024)
F = 256         # features
C = 8           # clusters
K = 5           # neighbors
H = 64          # hidden
O = 32          # output
RT = S // 128   # row tiles per core (8)
PCH = 2048      # psum chunk columns (4 banks)
NCH = B // PCH  # chunks per row tile (4)
NF = C + K + 4  # feat width incl. ones column (17)

_CACHE = {}


def _build():
    nc = bacc.Bacc("TRN2", target_bir_lowering=False, debug=False,
                   num_devices=NCORES)

    xTh_d = nc.dram_tensor("xTh", [128, 2, B], F16, kind="ExternalInput")
    xsTh_d = nc.dram_tensor("xsTh", [128, 2, S], F16, kind="ExternalInput")
    xs_d = nc.dram_tensor("xs", [S, F], F32, kind="ExternalInput")
    ccT_d = nc.dram_tensor("ccTa", [F, C], F32, kind="ExternalInput")
    cw_d = nc.dram_tensor("cw", [1, C], F32, kind="ExternalInput")
    temp_d = nc.dram_tensor("temp", [1, 1], F32, kind="ExternalInput")
    w1_d = nc.dram_tensor("W1a", [NF, H], F32, kind="ExternalInput")
    w2_d = nc.dram_tensor("W2a", [H + 1, O], F32, kind="ExternalInput")
    eye_d = nc.dram_tensor("eye", [128, 128], F32, kind="ExternalInput")

    enc_d = nc.dram_tensor("enc", [S, O], F32, kind="ExternalOutput")
    asn_d = nc.dram_tensor("asn", [S, C], F32, kind="ExternalOutput")
    knn_d = nc.dram_tensor("knn", [S, K], F32, kind="ExternalOutput")
    st_d = nc.dram_tensor("st", [S, 3], F32, kind="ExternalOutput")
    intra_d = nc.dram_tensor("intra", [1, 1], F32, kind="ExternalOutput")
    inter_d = nc.dram_tensor("inter", [1, 1], F32, kind="ExternalOutput")
    wu_d = nc.dram_tensor("wu", [1, 1], F32, kind="ExternalOutput")

    with tile.TileContext(nc) as tc:
        with (
            tc.tile_pool(name="big", bufs=1) as big,
            tc.tile_pool(name="sq", bufs=4) as sqp,
            tc.tile_pool(name="cst", bufs=1) as cst,
            tc.tile_pool(name="wk", bufs=2) as wk,
            tc.tile_pool(name="sm", bufs=4) as sm,
            tc.tile_pool(name="w8", bufs=8) as w8,
            tc.tile_pool(name="acc", bufs=2, space="PSUM") as pacc,
        ):
            # ---------- persistent tiles ----------
            xT_h = big.tile([128, 2, B], F16, tag="xT_h")
            r2row = big.tile([1, B], F16, tag="r2row")      # -r2/2
            xsT_h = big.tile([128, 2, S], F16, tag="xsT_h")
            intra_acc = big.tile([128, 1], F32, tag="intra_acc")
            top8A = big.tile([128, RT, 8], F32, tag="top8A")
            candA = big.tile([128, RT, NCH, 8], F32, tag="candA")
            featA = big.tile([128, RT, NF], F32, tag="featA")
            dcA = big.tile([128, RT, C], F32, tag="dcA")
            xrA = big.tile([128, RT, F], F32, tag="xrA")
            encA = big.tile([128, RT, O], F32, tag="encA")
            r2hA = big.tile([128, RT], F32, tag="r2hA")     # +r2/2 exact
            ssqA = big.tile([128, RT], F32, tag="ssqA")
            lsumA = big.tile([128, RT], F32, tag="lsumA")
            sexA = big.tile([128, RT], F32, tag="sexA")
            pxsA = big.tile([128, RT], F32, tag="pxsA")
            lnseA = big.tile([128, RT], F32, tag="lnseA")
            zA = big.tile([128, RT, C], F32, tag="zA")
            ezA = big.tile([128, RT, C], F32, tag="ezA")
            seA = big.tile([128, RT], F32, tag="seA")
            rseA = big.tile([128, RT], F32, tag="rseA")
            d25A = big.tile([128, RT, K], F32, tag="d25A")

            eye = cst.tile([128, 128], F32, tag="eye")
            ccT = cst.tile([128, 2, C], F32, tag="ccT")
            ccsq = cst.tile([128, 2, C], F32, tag="ccsq")
            ccT_h = cst.tile([128, 2, C], F16, tag="ccT_h")
            c2n_h = cst.tile([1, C], F16, tag="c2n_h")
            c2n = cst.tile([1, C], F32, tag="c2n")       # -c2/2
            c2p = cst.tile([1, C], F32, tag="c2p")       # +c2/2
            c2col = cst.tile([C, 1], F32, tag="c2col")   # +c2/2 column
            cwB = cst.tile([128, C], F32, tag="cwB")
            nIT = cst.tile([128, 1], F32, tag="nIT")     # -1/temp bcast
            w1 = cst.tile([NF, H], F32, tag="w1")
            w2 = cst.tile([H + 1, O], F32, tag="w2")
            cw_sb = cst.tile([1, C], F32, tag="cw_sb")
            t_sb = cst.tile([1, 1], F32, tag="t_sb")
            nrT = cst.tile([1, 1], F32, tag="nrT")
            ones_f16 = cst.tile([128, 1], F16, tag="ones_f16")
            one_f16r = cst.tile([1, 128], F16, tag="one_f16r")
            ones1 = cst.tile([1, 128], F32, tag="ones1")
            ones_col = cst.tile([128, 1], F32, tag="ones_col")

            # ---------- input DMAs (fast path first on sync queue) -------
            nc.sync.dma_start(xsT_h[:], xsTh_d.ap())
            for p in range(NCH):
                psl = slice(p * PCH, (p + 1) * PCH)
                nc.sync.dma_start(xT_h[:, :, psl], xTh_d.ap()[:, :, psl])
            # Secondary inputs share the same (serial) queue AFTER the xT
            # pieces so they don't steal HBM bandwidth from the matmul-
            # gating loads during the 8-core startup window.
            nc.sync.dma_start(
                xrA[:], xs_d.ap().rearrange("(t p) f -> p t f", p=128))
            for h in range(2):
                nc.sync.dma_start(ccT[:, h, :],
                                  ccT_d.ap()[h * 128:(h + 1) * 128, :])
            nc.sync.dma_start(cw_sb[:], cw_d.ap())
            nc.sync.dma_start(t_sb[:], temp_d.ap())
            nc.sync.dma_start(w1[:], w1_d.ap())
            nc.sync.dma_start(w2[:], w2_d.ap())
            nc.sync.dma_start(eye[:], eye_d.ap())
            nc.vector.memset(ones_f16[:], 1.0)
            nc.vector.memset(one_f16r[:], 1.0)
            nc.vector.memset(ones1[:], 1.0)
            nc.vector.memset(ones_col[:], 1.0)

            # PE warm-up during the input-DMA window: dense dummy matmuls
            # keep the HAM clock-gate busy so real matmuls start at 2.4 GHz.
            wu_s = cst.tile([128, 512], F16, tag="wu_s")
            nc.vector.memset(wu_s[:], 0.001)
            wup = pacc.tile([1, 512], F32, tag="acc")
            for i in range(30):
                nc.tensor.matmul(wup[:], ones_f16[:], wu_s[:],
                                 start=(i == 0), stop=(i == 29))
            wu_sb = sm.tile([1, 512], F32, tag="wu_sb")
            nc.scalar.copy(wu_sb[:], wup[:])
            wu_r = sm.tile([1, 1], F32, tag="wu_r")
            nc.vector.reduce_sum(wu_r[:], wu_sb[:], axis=AX.X)
            nc.gpsimd.dma_start(wu_d.ap(), wu_r[:])

            # ======== PHASE A: setup pieces then dense matmul stream =====
            def emit_chunk(t, c):
                tsl = slice(t * 128, (t + 1) * 128)
                acc = pacc.tile([128, PCH], F32, tag="acc")
                for k in range(2):
                    for n in range(PCH // 512):
                        csl = slice(c * PCH + n * 512, c * PCH + (n + 1) * 512)
                        nsl = slice(n * 512, (n + 1) * 512)
                        nc.tensor.matmul(acc[:, nsl], xsT_h[:, k, tsl],
                                         xT_h[:, k, csl],
                                         start=(k == 0), stop=False)
                for n in range(PCH // 512):
                    csl = slice(c * PCH + n * 512, c * PCH + (n + 1) * 512)
                    nsl = slice(n * 512, (n + 1) * 512)
                    nc.tensor.matmul(acc[:, nsl], one_f16r[:], r2row[:, csl],
                                     start=False, stop=True)
                nc.vector.max(candA[:, t, c, :], acc[:])

            sq0s, sq1s = [], []
            for p in range(NCH):
                # squares for cols [p*2048,(p+1)*2048), emitted upfront so
                # c-row starts never wait on the ACT queue
                psl = slice(p * PCH, (p + 1) * PCH)
                sq0 = sqp.tile([128, PCH], F16, tag="sq0")
                nc.vector.tensor_mul(sq0[:], xT_h[:, 0, psl], xT_h[:, 0, psl])
                sq0s.append(sq0)
                sq1 = sqp.tile([128, PCH], F16, tag="sq1")
                nc.scalar.square(sq1[:], xT_h[:, 1, psl])
                sq1s.append(sq1)

            def emit_piece(p):
                # -r2/2 row for cols [p*2048,(p+1)*2048)
                for r in range(PCH // 512):
                    gsl = slice(p * PCH + r * 512, p * PCH + (r + 1) * 512)
                    lsl = slice(r * 512, (r + 1) * 512)
                    rp = pacc.tile([1, 512], F32, tag="acc")
                    nc.tensor.matmul(rp[:], ones_f16[:], sq0s[p][:, lsl],
                                     start=True, stop=False)
                    nc.tensor.matmul(rp[:], ones_f16[:], sq1s[p][:, lsl],
                                     start=False, stop=True)
                    nc.scalar.mul(r2row[0:1, gsl], rp[:], -0.5)

            def emit_cluster_setup():
                nc.vector.tensor_mul(ccsq[:], ccT[:], ccT[:])
                c2ps = pacc.tile([1, C], F32, tag="acc")
                for h in range(2):
                    nc.tensor.matmul(c2ps[:], ones_col[:], ccsq[:, h, :],
                                     start=(h == 0), stop=(h == 1))
                nc.scalar.mul(c2n[:], c2ps[:], -0.5)
                nc.vector.tensor_copy(ccT_h[:], ccT[:])
                nc.vector.tensor_copy(c2n_h[:], c2n[:])
                nc.scalar.mul(c2p[:], c2ps[:], 0.5)
                ccp = pacc.tile([C, 1], F32, tag="acc")
                nc.tensor.matmul(ccp[:], c2p[:], ones_col[0:1, 0:1],
                                 start=True, stop=True)
                nc.vector.tensor_copy(c2col[:], ccp[:])
                rT = sm.tile([1, 1], F32, tag="s1")
                nc.vector.reciprocal(rT[:], t_sb[:])
                nc.vector.tensor_scalar_mul(nrT[:], rT[:], -1.0)
                bp = pacc.tile([128, 1], F32, tag="acc")
                nc.tensor.matmul(bp[:], ones1[:], nrT[:],
                                 start=True, stop=True)
                nc.vector.tensor_copy(nIT[:], bp[:])
                cp = pacc.tile([128, C], F32, tag="acc")
                nc.tensor.matmul(cp[:], ones1[:], cw_sb[:],
                                 start=True, stop=True)
                nc.vector.tensor_copy(cwB[:], cp[:])

            pcSs = []

            def emit_dc():
                for t in range(RT):
                    tsl = slice(t * 128, (t + 1) * 128)
                    pc = pacc.tile([128, C], F32, tag="acc")
                    for k in range(2):
                        nc.tensor.matmul(pc[:], xsT_h[:, k, tsl],
                                         ccT_h[:, k, :],
                                         start=(k == 0), stop=False)
                    nc.tensor.matmul(pc[:], one_f16r[:], c2n_h[:],
                                     start=False, stop=True)
                    pcS = w8.tile([128, C], F32, tag="pcS")
                    nc.scalar.copy(pcS[:], pc[:])
                    pcSs.append(pcS)

            exTs = []
            pxTs = []

            def emit_stats_phase(ph):
                # Local stats via ACT accumulators + gpsimd multiplies,
                # spread across c-rows so ACT never blocks piece setup.
                if ph == 0:
                    for t in range(RT):
                        j1 = wk.tile([128, F], F32, tag="j1")
                        nc.scalar.activation(j1[:], xrA[:, t, :], ACTF.Square,
                                             accum_out=ssqA[:, t:t + 1])
                elif ph == 1:
                    for t in range(RT):
                        exT = w8.tile([128, F], F32, tag="exT")
                        nc.scalar.activation(exT[:], xrA[:, t, :], ACTF.Exp,
                                             accum_out=sexA[:, t:t + 1])
                        exTs.append(exT)
                elif ph == 2:
                    for t in range(RT):
                        pxT = w8.tile([128, F], F32, tag="pxT")
                        nc.gpsimd.tensor_mul(pxT[:], exTs[t][:], xrA[:, t, :])
                        pxTs.append(pxT)
                    for t in range(RT):
                        j3 = wk.tile([128, F], F32, tag="j3")
                        nc.scalar.activation(j3[:], pxTs[t][:], ACTF.Copy,
                                             accum_out=pxsA[:, t:t + 1])
                else:
                    for t in range(RT):
                        j2 = wk.tile([128, F], F32, tag="j2")
                        nc.scalar.activation(j2[:], xrA[:, t, :], ACTF.Copy,
                                             accum_out=lsumA[:, t:t + 1])

            for c in range(NCH):
                emit_piece(c)
                for t in range(RT):
                    emit_chunk(t, c)
                    if c == 0 and t == 1:
                        emit_cluster_setup()
                    if c == 1 and t == 3:
                        for _ph in range(4):
                            emit_stats_phase(_ph)
                    if c == 1 and t == 6:
                        emit_dc()
            for t in range(RT):
                nc.vector.max(top8A[:, t, :],
                              candA[:, t, :, :].rearrange("p a b -> p (a b)"))
            # --- inter-cluster loss (identical on all cores) ---
            g8 = pacc.tile([C, C], F32, tag="acc")
            for h in range(2):
                nc.tensor.matmul(g8[:], ccT[:, h, :], ccT[:, h, :],
                                 start=(h == 0), stop=False)
            nc.tensor.matmul(g8[:], ones1[0:1, 0:C], c2n[:],
                             start=False, stop=True)
            d2cc = sm.tile([C, C], F32, tag="d2cc")
            nc.vector.tensor_scalar(d2cc[:], g8[:], c2col[:], -2.0,
                                    op0=ALU.subtract, op1=ALU.mult)
            nc.vector.tensor_scalar_max(d2cc[:], d2cc[:], 1e-12)
            ccd = sm.tile([C, C], F32, tag="ccd")
            nc.scalar.sqrt(ccd[:], d2cc[:])
            crs = sm.tile([C, 1], F32, tag="crs")
            nc.vector.reduce_sum(crs[:], ccd[:], axis=AX.X)
            ip8 = pacc.tile([1, 1], F32, tag="acc")
            nc.tensor.matmul(ip8[:], crs[:], ones_col[0:C, 0:1],
                             start=True, stop=True)
            inter_sb = sm.tile([1, 1], F32, tag="s1b")
            nc.scalar.mul(inter_sb[:], ip8[:], 1.0 / (C * (C - 1)))
            nc.gpsimd.dma_start(inter_d.ap(), inter_sb[:])


            # ================= PHASE B: post-processing ==================
            nc.vector.tensor_scalar_mul(
                featA[:, :, C + K:C + K + 1].rearrange("p t o -> p (t o)"),
                lsumA[:], 1.0 / F)
            lm2 = sm.tile([128, RT], F32, tag="lm2")
            nc.vector.tensor_mul(
                lm2[:],
                featA[:, :, C + K:C + K + 1].rearrange("p t o -> p (t o)"),
                featA[:, :, C + K:C + K + 1].rearrange("p t o -> p (t o)"))
            v1 = sm.tile([128, RT], F32, tag="v1")
            nc.vector.scalar_tensor_tensor(
                out=v1[:], in0=lm2[:], scalar=-float(F), in1=ssqA[:],
                op0=ALU.mult, op1=ALU.add)
            nc.vector.tensor_scalar_mul(v1[:], v1[:], 1.0 / (F - 1))
            rs2 = sm.tile([128, RT], F32, tag="rs2")
            nc.vector.reciprocal(rs2[:], sexA[:])
            t1 = sm.tile([128, RT], F32, tag="t1")
            nc.vector.tensor_mul(t1[:], pxsA[:], rs2[:])

            # --- B1b: cluster d^2 from drained matmul results ---
            nc.vector.tensor_scalar_mul(r2hA[:], ssqA[:], 0.5)
            for t in range(RT):
                nc.vector.tensor_scalar(dcA[:, t, :], pcSs[t][:],
                                        r2hA[:, t:t + 1], -2.0,
                                        op0=ALU.subtract, op1=ALU.mult)
            nc.vector.tensor_scalar_max(dcA[:], dcA[:], 1e-12)

            # --- B2: knn d^2 ---
            nc.vector.tensor_sub(
                d25A[:], top8A[:, :, 1:1 + K],
                r2hA[:].unsqueeze(2).to_broadcast([128, RT, K]))
            nc.vector.tensor_scalar(d25A[:], d25A[:], -2.0, None, op0=ALU.mult)
            nc.vector.tensor_scalar_max(d25A[:], d25A[:], 1e-12)

            # --- B3: sqrt group (std, knn, dc) ---
            sd = sm.tile([128, RT], F32, tag="sd")
            nc.scalar.sqrt(sd[:], v1[:])
            nc.vector.tensor_scalar_add(
                featA[:, :, C + K + 1:C + K + 2].rearrange("p t o -> p (t o)"),
                sd[:], 1e-8)
            nc.scalar.sqrt(featA[:, :, C:C + K], d25A[:])
            nc.scalar.sqrt(dcA[:], dcA[:])

            # --- B4: softmax assign + intra (no max-subtraction) ---
            nc.vector.tensor_scalar_mul(
                zA[:].rearrange("p t c -> p (t c)"),
                dcA[:].rearrange("p t c -> p (t c)"), nIT[:])
            nc.scalar.activation(ezA[:], zA[:], ACTF.Exp)
            nc.vector.reduce_sum(seA[:], ezA[:], axis=AX.X)
            nc.vector.reciprocal(rseA[:], seA[:])
            nc.vector.tensor_mul(
                ezA[:], ezA[:],
                rseA[:].unsqueeze(2).to_broadcast([128, RT, C]))
            nc.vector.tensor_mul(
                featA[:, :, 0:C], ezA[:],
                cwB[:].unsqueeze(1).to_broadcast([128, RT, C]))
            dxa = wk.tile([128, RT, C], F32, tag="dxa")
            nc.vector.tensor_mul(dxa[:], dcA[:], featA[:, :, 0:C])
            nc.vector.tensor_reduce(intra_acc[:], dxa[:], axis=AX.XY,
                                    op=ALU.add)

            # --- entropy tail (Ln table last) ---
            nc.scalar.activation(lnseA[:], sexA[:], ACTF.Ln)
            nc.vector.tensor_sub(
                featA[:, :, C + K + 2:C + K + 3].rearrange("p t o -> p (t o)"),
                lnseA[:], t1[:])

            # --- B5: MLP ---
            # Layer 1 in fp16 (transpose is 1 matmul instead of an fp32
            # LOW/HIGH pair); layer 2 stays fp32. Loops are split per op
            # so the PE/ACT/DVE stages pipeline instead of ping-ponging.
            nc.vector.memset(featA[:, :, NF - 1:NF], 1.0)
            feat_h = big.tile([128, RT, NF], F16, tag="feat_h")
            nc.vector.tensor_copy(feat_h[:], featA[:])
            eye_h = cst.tile([128, 128], F16, tag="eye_h")
            nc.vector.tensor_copy(eye_h[:], eye[:])
            w1_h = cst.tile([NF, H], F16, tag="w1_h")
            nc.vector.tensor_copy(w1_h[:], w1[:])
            fThs = []
            for t in range(RT):
                fTp = pacc.tile([NF, 128], F16, tag="acc")
                nc.tensor.matmul(fTp[:], feat_h[:, t, :], eye_h[:],
                                 is_transpose=True, start=True, stop=True)
                fTh = w8.tile([NF, 128], F16, tag="fTh")
                nc.scalar.copy(fTh[:], fTp[:])
                fThs.append(fTh)
            hraTs = []
            for t in range(RT):
                hTp = pacc.tile([H, 128], F32, tag="acc")
                nc.tensor.matmul(hTp[:], w1_h[:], fThs[t][:],
                                 start=True, stop=True)
                hraT = w8.tile([H + 1, 128], F32, tag="hraT")
                nc.vector.tensor_scalar_max(hraT[0:H, :], hTp[:], 0.0)
                nc.vector.memset(hraT[H:H + 1, :], 1.0)
                hraTs.append(hraT)
            for t in range(RT):
                ep = pacc.tile([128, O], F32, tag="acc")
                nc.tensor.matmul(ep[:], hraTs[t][:], w2[:],
                                 start=True, stop=True)
                nc.scalar.copy(encA[:, t, :], ep[:])

            # ---------- batched output DMAs ----------
            nc.gpsimd.dma_start(
                knn_d.ap().rearrange("(t p) k -> p t k", p=128),
                featA[:, :, C:C + K])
            nc.gpsimd.dma_start(
                asn_d.ap().rearrange("(t p) c -> p t c", p=128),
                featA[:, :, 0:C])
            nc.gpsimd.dma_start(
                st_d.ap().rearrange("(t p) s -> p t s", p=128),
                featA[:, :, C + K:C + K + 3])
            nc.gpsimd.dma_start(
                enc_d.ap().rearrange("(t p) o -> p t o", p=128), encA[:])

            # ---------- intra partial reduce ----------
            ipp = pacc.tile([1, 1], F32, tag="acc")
            nc.tensor.matmul(ipp[:], intra_acc[:], ones_col[:],
                             start=True, stop=True)
            intra_sb = sm.tile([1, 1], F32, tag="s1c")
            nc.scalar.mul(intra_sb[:], ipp[:], 1.0 / (B * C))
            nc.gpsimd.dma_start(intra_d.ap(), intra_sb[:])

    nc.compile()
    return nc


def kernel(x, cluster_centers, temperature, cluster_weights, W1, b1, W2, b2,
           noise):
    del noise  # tie-break only; cannot change output values beyond ~1e-6
    x = np.asarray(x, dtype=np.float32)
    cc = np.asarray(cluster_centers, dtype=np.float32)
    temp = np.asarray(temperature, dtype=np.float32).reshape(1, 1)
    cw = np.asarray(cluster_weights, dtype=np.float32).reshape(1, C)
    W1 = np.asarray(W1, dtype=np.float32)
    b1 = np.asarray(b1, dtype=np.float32)
    W2 = np.asarray(W2, dtype=np.float32)
    b2 = np.asarray(b2, dtype=np.float32)

    if "nc" not in _CACHE:
        _CACHE["nc"] = _build()
    nc = _CACHE["nc"]

    xT = np.ascontiguousarray(x.T)                      # [256, 8192]
    f16 = np.float16
    xTh = np.ascontiguousarray(
        xT.reshape(2, 128, B).transpose(1, 0, 2)).astype(f16)
    ccT = np.ascontiguousarray(cc.T)
    W1a = np.concatenate([W1, b1.reshape(1, H)], axis=0)   # [17, 64]
    W2a = np.concatenate([W2, b2.reshape(1, O)], axis=0)
    eye = np.eye(128, dtype=np.float32)

    in_maps = []
    for c in range(NCORES):
        sl = slice(c * S, (c + 1) * S)
        xsT = np.ascontiguousarray(xT[:, sl])
        xsTh = np.ascontiguousarray(
            xsT.reshape(2, 128, S).transpose(1, 0, 2)).astype(f16)
        in_maps.append({
            "xTh": xTh,
            "xsTh": xsTh,
            "xs": np.ascontiguousarray(x[sl]),
            "ccTa": ccT,
            "cw": cw,
            "temp": temp,
            "W1a": W1a,
            "W2a": W2a,
            "eye": eye,
        })

    res = run_bass_kernel_spmd(nc, in_maps, core_ids=list(range(NCORES)))
    rs = res.results

    encoded = np.concatenate([r["enc"] for r in rs], axis=0)
    assign = np.concatenate([r["asn"] for r in rs], axis=0)
    knn = np.concatenate([r["knn"] for r in rs], axis=0)
    stats = np.concatenate([r["st"] for r in rs], axis=0)
    intra = np.float32(sum(float(r["intra"][0, 0]) for r in rs))
    inter = np.float32(rs[0]["inter"][0, 0])
    loss = np.float32(intra - 0.1 * inter)
    return encoded, assign, knn, stats, loss


# revision 28
# speedup vs baseline: 1.0319x; 1.0319x over previous
"""Trainium2 Bass kernel for AdvancedWeightPredictorNetwork (retrieval_knn).

Strategy (8 NeuronCores, data-parallel over rows of x):
  - Each core owns a 1024-row shard of x [8192, 256]; x^T is replicated
    (pre-laid-out in fp16 by the host as part of sharding).
  - Distance ranking: G' = x_shard @ x_all^T - r2_all/2 via fp16 matmuls
    into PSUM (k-chunks: two 128-feature halves + one row carrying
    -|x_j|^2/2, so ranking by G' equals ranking by -cdist^2).
    Per 2048-col PSUM chunk, vector.max (top-8, sorted desc) reduces
    the chunk straight from PSUM; a merge max over 4x8 candidates gives
    the global top-8 per row. Entry 0 is always the row itself; entries
    1..5 are the 5 nearest: knn = sqrt(r2_i - 2*s) with exact fp32
    r2_i (= sum x^2 from the local-stats pass). The tie-break noise
    matrix only permutes neighbors whose distances agree to ~1e-6 (far
    below tolerance), so it is not streamed.
  - Phase A is a pure dense matmul stream (PE clock-gate stays warm);
    phase B (cluster softmax / stats / MLP) is batched per op across
    row tiles to minimize ACT table reloads and DVE op count. exp() is
    applied without max-subtraction: inputs are bounded (|x| < 6,
    logits in [-25, 0]) so fp32 exp is safe and matches softmax math.
  - Scalar losses: per-core intra partial + inter computed on device;
    host sums the 8 partials.
"""

import numpy as np

import concourse.bacc as bacc
import concourse.tile as tile
import concourse.mybir as mybir
from concourse.bass_utils import run_bass_kernel_spmd

F32 = mybir.dt.float32
F16 = mybir.dt.float16
ALU = mybir.AluOpType
ACTF = mybir.ActivationFunctionType
AX = mybir.AxisListType

B = 8192        # total rows
NCORES = 8
S = B // NCORES  # rows per core (1024)
F = 256         # features
C = 8           # clusters
K = 5           # neighbors
H = 64          # hidden
O = 32          # output
RT = S // 128   # row tiles per core (8)
PCH = 2048      # psum chunk columns (4 banks)
NCH = B // PCH  # chunks per row tile (4)
NF = C + K + 4  # feat width incl. ones column (17)

_CACHE = {}


def _build():
    nc = bacc.Bacc("TRN2", target_bir_lowering=False, debug=False,
                   num_devices=NCORES)

    xTh_d = nc.dram_tensor("xTh", [128, 2, B], F16, kind="ExternalInput")
    xsTh_d = nc.dram_tensor("xsTh", [128, 2, S], F16, kind="ExternalInput")
    xs_d = nc.dram_tensor("xs", [S, F], F32, kind="ExternalInput")
    ccT_d = nc.dram_tensor("ccTa", [F, C], F32, kind="ExternalInput")
    cw_d = nc.dram_tensor("cw", [1, C], F32, kind="ExternalInput")
    temp_d = nc.dram_tensor("temp", [1, 1], F32, kind="ExternalInput")
    w1_d = nc.dram_tensor("W1a", [NF, H], F32, kind="ExternalInput")
    w2_d = nc.dram_tensor("W2a", [H + 1, O], F32, kind="ExternalInput")
    eye_d = nc.dram_tensor("eye", [128, 128], F32, kind="ExternalInput")

    enc_d = nc.dram_tensor("enc", [S, O], F32, kind="ExternalOutput")
    asn_d = nc.dram_tensor("asn", [S, C], F32, kind="ExternalOutput")
    knn_d = nc.dram_tensor("knn", [S, K], F32, kind="ExternalOutput")
    st_d = nc.dram_tensor("st", [S, 3], F32, kind="ExternalOutput")
    intra_d = nc.dram_tensor("intra", [1, 1], F32, kind="ExternalOutput")
    inter_d = nc.dram_tensor("inter", [1, 1], F32, kind="ExternalOutput")
    wu_d = nc.dram_tensor("wu", [1, 1], F32, kind="ExternalOutput")

    with tile.TileContext(nc) as tc:
        with (
            tc.tile_pool(name="big", bufs=1) as big,
            tc.tile_pool(name="sq", bufs=2) as sqp,
            tc.tile_pool(name="cst", bufs=1) as cst,
            tc.tile_pool(name="wk", bufs=2) as wk,
            tc.tile_pool(name="sm", bufs=4) as sm,
            tc.tile_pool(name="w8", bufs=8) as w8,
            tc.tile_pool(name="acc", bufs=2, space="PSUM") as pacc,
        ):
            # ---------- persistent tiles ----------
            xT_h = big.tile([128, 2, B], F16, tag="xT_h")
            r2row = big.tile([1, B], F16, tag="r2row")      # -r2/2
            xsT_h = big.tile([128, 2, S], F16, tag="xsT_h")
            intra_acc = big.tile([128, 1], F32, tag="intra_acc")
            top8A = big.tile([128, RT, 8], F32, tag="top8A")
            candA = big.tile([128, RT, NCH, 8], F32, tag="candA")
            featA = big.tile([128, RT, NF], F32, tag="featA")
            dcA = big.tile([128, RT, C], F32, tag="dcA")
            xrA = big.tile([128, RT, F], F32, tag="xrA")
            encA = big.tile([128, RT, O], F32, tag="encA")
            r2hA = big.tile([128, RT], F32, tag="r2hA")     # +r2/2 exact
            ssqA = big.tile([128, RT], F32, tag="ssqA")
            lsumA = big.tile([128, RT], F32, tag="lsumA")
            sexA = big.tile([128, RT], F32, tag="sexA")
            pxsA = big.tile([128, RT], F32, tag="pxsA")
            lnseA = big.tile([128, RT], F32, tag="lnseA")
            zA = big.tile([128, RT, C], F32, tag="zA")
            ezA = big.tile([128, RT, C], F32, tag="ezA")
            seA = big.tile([128, RT], F32, tag="seA")
            rseA = big.tile([128, RT], F32, tag="rseA")
            d25A = big.tile([128, RT, K], F32, tag="d25A")

            eye = cst.tile([128, 128], F32, tag="eye")
            ccT = cst.tile([128, 2, C], F32, tag="ccT")
            ccsq = cst.tile([128, 2, C], F32, tag="ccsq")
            ccT_h = cst.tile([128, 2, C], F16, tag="ccT_h")
            c2n_h = cst.tile([1, C], F16, tag="c2n_h")
            c2n = cst.tile([1, C], F32, tag="c2n")       # -c2/2
            c2p = cst.tile([1, C], F32, tag="c2p")       # +c2/2
            c2col = cst.tile([C, 1], F32, tag="c2col")   # +c2/2 column
            cwB = cst.tile([128, C], F32, tag="cwB")
            nIT = cst.tile([128, 1], F32, tag="nIT")     # -1/temp bcast
            w1 = cst.tile([NF, H], F32, tag="w1")
            w2 = cst.tile([H + 1, O], F32, tag="w2")
            cw_sb = cst.tile([1, C], F32, tag="cw_sb")
            t_sb = cst.tile([1, 1], F32, tag="t_sb")
            nrT = cst.tile([1, 1], F32, tag="nrT")
            ones_f16 = cst.tile([128, 1], F16, tag="ones_f16")
            one_f16r = cst.tile([1, 128], F16, tag="one_f16r")
            ones1 = cst.tile([1, 128], F32, tag="ones1")
            ones_col = cst.tile([128, 1], F32, tag="ones_col")

            # ---------- input DMAs (fast path first on sync queue) -------
            nc.sync.dma_start(xsT_h[:], xsTh_d.ap())
            for p in range(NCH):
                psl = slice(p * PCH, (p + 1) * PCH)
                nc.sync.dma_start(xT_h[:, :, psl], xTh_d.ap()[:, :, psl])
            # Secondary inputs share the same (serial) queue AFTER the xT
            # pieces so they don't steal HBM bandwidth from the matmul-
            # gating loads during the 8-core startup window.
            nc.sync.dma_start(
                xrA[:], xs_d.ap().rearrange("(t p) f -> p t f", p=128))
            for h in range(2):
                nc.sync.dma_start(ccT[:, h, :],
                                  ccT_d.ap()[h * 128:(h + 1) * 128, :])
            nc.sync.dma_start(cw_sb[:], cw_d.ap())
            nc.sync.dma_start(t_sb[:], temp_d.ap())
            nc.sync.dma_start(w1[:], w1_d.ap())
            nc.sync.dma_start(w2[:], w2_d.ap())
            nc.sync.dma_start(eye[:], eye_d.ap())
            nc.vector.memset(ones_f16[:], 1.0)
            nc.vector.memset(one_f16r[:], 1.0)
            nc.vector.memset(ones1[:], 1.0)
            nc.vector.memset(ones_col[:], 1.0)

            # PE warm-up during the input-DMA window: dense dummy matmuls
            # keep the HAM clock-gate busy so real matmuls start at 2.4 GHz.
            wu_s = cst.tile([128, 512], F16, tag="wu_s")
            nc.vector.memset(wu_s[:], 0.001)
            wup = pacc.tile([1, 512], F32, tag="acc")
            for i in range(30):
                nc.tensor.matmul(wup[:], ones_f16[:], wu_s[:],
                                 start=(i == 0), stop=(i == 29))
            wu_sb = sm.tile([1, 512], F32, tag="wu_sb")
            nc.scalar.copy(wu_sb[:], wup[:])
            wu_r = sm.tile([1, 1], F32, tag="wu_r")
            nc.vector.reduce_sum(wu_r[:], wu_sb[:], axis=AX.X)
            nc.gpsimd.dma_start(wu_d.ap(), wu_r[:])

            # ======== PHASE A: setup pieces then dense matmul stream =====
            def emit_chunk(t, c):
                tsl = slice(t * 128, (t + 1) * 128)
                acc = pacc.tile([128, PCH], F32, tag="acc")
                for k in range(2):
                    for n in range(PCH // 512):
                        csl = slice(c * PCH + n * 512, c * PCH + (n + 1) * 512)
                        nsl = slice(n * 512, (n + 1) * 512)
                        nc.tensor.matmul(acc[:, nsl], xsT_h[:, k, tsl],
                                         xT_h[:, k, csl],
                                         start=(k == 0), stop=False)
                for n in range(PCH // 512):
                    csl = slice(c * PCH + n * 512, c * PCH + (n + 1) * 512)
                    nsl = slice(n * 512, (n + 1) * 512)
                    nc.tensor.matmul(acc[:, nsl], one_f16r[:], r2row[:, csl],
                                     start=False, stop=True)
                nc.vector.max(candA[:, t, c, :], acc[:])

            def emit_piece(p):
                # piece p: squares + -r2/2 row for cols [p*2048,(p+1)*2048)
                psl = slice(p * PCH, (p + 1) * PCH)
                sq0 = sqp.tile([128, PCH], F16, tag="sq0")
                nc.vector.tensor_mul(sq0[:], xT_h[:, 0, psl], xT_h[:, 0, psl])
                sq1 = sqp.tile([128, PCH], F16, tag="sq1")
                nc.scalar.square(sq1[:], xT_h[:, 1, psl])
                for r in range(PCH // 512):
                    gsl = slice(p * PCH + r * 512, p * PCH + (r + 1) * 512)
                    lsl = slice(r * 512, (r + 1) * 512)
                    rp = pacc.tile([1, 512], F32, tag="acc")
                    nc.tensor.matmul(rp[:], ones_f16[:], sq0[:, lsl],
                                     start=True, stop=False)
                    nc.tensor.matmul(rp[:], ones_f16[:], sq1[:, lsl],
                                     start=False, stop=True)
                    nc.scalar.mul(r2row[0:1, gsl], rp[:], -0.5)

            def emit_cluster_setup():
                nc.vector.tensor_mul(ccsq[:], ccT[:], ccT[:])
                c2ps = pacc.tile([1, C], F32, tag="acc")
                for h in range(2):
                    nc.tensor.matmul(c2ps[:], ones_col[:], ccsq[:, h, :],
                                     start=(h == 0), stop=(h == 1))
                nc.scalar.mul(c2n[:], c2ps[:], -0.5)
                nc.vector.tensor_copy(ccT_h[:], ccT[:])
                nc.vector.tensor_copy(c2n_h[:], c2n[:])
                nc.scalar.mul(c2p[:], c2ps[:], 0.5)
                ccp = pacc.tile([C, 1], F32, tag="acc")
                nc.tensor.matmul(ccp[:], c2p[:], ones_col[0:1, 0:1],
                                 start=True, stop=True)
                nc.vector.tensor_copy(c2col[:], ccp[:])
                rT = sm.tile([1, 1], F32, tag="s1")
                nc.vector.reciprocal(rT[:], t_sb[:])
                nc.vector.tensor_scalar_mul(nrT[:], rT[:], -1.0)
                bp = pacc.tile([128, 1], F32, tag="acc")
                nc.tensor.matmul(bp[:], ones1[:], nrT[:],
                                 start=True, stop=True)
                nc.vector.tensor_copy(nIT[:], bp[:])
                cp = pacc.tile([128, C], F32, tag="acc")
                nc.tensor.matmul(cp[:], ones1[:], cw_sb[:],
                                 start=True, stop=True)
                nc.vector.tensor_copy(cwB[:], cp[:])

            def emit_dc():
                nc.vector.tensor_scalar_mul(r2hA[:], ssqA[:], 0.5)
                for t in range(RT):
                    tsl = slice(t * 128, (t + 1) * 128)
                    pc = pacc.tile([128, C], F32, tag="acc")
                    for k in range(2):
                        nc.tensor.matmul(pc[:], xsT_h[:, k, tsl],
                                         ccT_h[:, k, :],
                                         start=(k == 0), stop=False)
                    nc.tensor.matmul(pc[:], one_f16r[:], c2n_h[:],
                                     start=False, stop=True)
                    nc.vector.tensor_scalar(dcA[:, t, :], pc[:],
                                            r2hA[:, t:t + 1], -2.0,
                                            op0=ALU.subtract, op1=ALU.mult)
                nc.vector.tensor_scalar_max(dcA[:], dcA[:], 1e-12)

            exTs = []
            pxTs = []

            def emit_stats_phase(ph):
                # Local stats via ACT accumulators + gpsimd multiplies,
                # spread across c-rows so ACT never blocks piece setup.
                if ph == 0:
                    for t in range(RT):
                        j1 = wk.tile([128, F], F32, tag="j1")
                        nc.scalar.activation(j1[:], xrA[:, t, :], ACTF.Square,
                                             accum_out=ssqA[:, t:t + 1])
                elif ph == 1:
                    for t in range(RT):
                        exT = w8.tile([128, F], F32, tag="exT")
                        nc.scalar.activation(exT[:], xrA[:, t, :], ACTF.Exp,
                                             accum_out=sexA[:, t:t + 1])
                        exTs.append(exT)
                elif ph == 2:
                    for t in range(RT):
                        pxT = w8.tile([128, F], F32, tag="pxT")
                        nc.gpsimd.tensor_mul(pxT[:], exTs[t][:], xrA[:, t, :])
                        pxTs.append(pxT)
                    for t in range(RT):
                        j3 = wk.tile([128, F], F32, tag="j3")
                        nc.scalar.activation(j3[:], pxTs[t][:], ACTF.Copy,
                                             accum_out=pxsA[:, t:t + 1])
                else:
                    for t in range(RT):
                        j2 = wk.tile([128, F], F32, tag="j2")
                        nc.scalar.activation(j2[:], xrA[:, t, :], ACTF.Copy,
                                             accum_out=lsumA[:, t:t + 1])

            for c in range(NCH):
                emit_piece(c)
                for t in range(RT):
                    emit_chunk(t, c)
                    if c == 0 and t == 1:
                        emit_cluster_setup()
                    if c == 0 and t == 3:
                        for _ph in range(4):
                            emit_stats_phase(_ph)
                    if c == 1 and t == 6:
                        emit_dc()
            for t in range(RT):
                nc.vector.max(top8A[:, t, :],
                              candA[:, t, :, :].rearrange("p a b -> p (a b)"))
            # --- inter-cluster loss (identical on all cores) ---
            g8 = pacc.tile([C, C], F32, tag="acc")
            for h in range(2):
                nc.tensor.matmul(g8[:], ccT[:, h, :], ccT[:, h, :],
                                 start=(h == 0), stop=False)
            nc.tensor.matmul(g8[:], ones1[0:1, 0:C], c2n[:],
                             start=False, stop=True)
            d2cc = sm.tile([C, C], F32, tag="d2cc")
            nc.vector.tensor_scalar(d2cc[:], g8[:], c2col[:], -2.0,
                                    op0=ALU.subtract, op1=ALU.mult)
            nc.vector.tensor_scalar_max(d2cc[:], d2cc[:], 1e-12)
            ccd = sm.tile([C, C], F32, tag="ccd")
            nc.scalar.sqrt(ccd[:], d2cc[:])
            crs = sm.tile([C, 1], F32, tag="crs")
            nc.vector.reduce_sum(crs[:], ccd[:], axis=AX.X)
            ip8 = pacc.tile([1, 1], F32, tag="acc")
            nc.tensor.matmul(ip8[:], crs[:], ones_col[0:C, 0:1],
                             start=True, stop=True)
            inter_sb = sm.tile([1, 1], F32, tag="s1b")
            nc.scalar.mul(inter_sb[:], ip8[:], 1.0 / (C * (C - 1)))
            nc.gpsimd.dma_start(inter_d.ap(), inter_sb[:])


            # ================= PHASE B: post-processing ==================
            nc.vector.tensor_scalar_mul(
                featA[:, :, C + K:C + K + 1].rearrange("p t o -> p (t o)"),
                lsumA[:], 1.0 / F)
            lm2 = sm.tile([128, RT], F32, tag="lm2")
            nc.vector.tensor_mul(
                lm2[:],
                featA[:, :, C + K:C + K + 1].rearrange("p t o -> p (t o)"),
                featA[:, :, C + K:C + K + 1].rearrange("p t o -> p (t o)"))
            v1 = sm.tile([128, RT], F32, tag="v1")
            nc.vector.scalar_tensor_tensor(
                out=v1[:], in0=lm2[:], scalar=-float(F), in1=ssqA[:],
                op0=ALU.mult, op1=ALU.add)
            nc.vector.tensor_scalar_mul(v1[:], v1[:], 1.0 / (F - 1))
            rs2 = sm.tile([128, RT], F32, tag="rs2")
            nc.vector.reciprocal(rs2[:], sexA[:])
            t1 = sm.tile([128, RT], F32, tag="t1")
            nc.vector.tensor_mul(t1[:], pxsA[:], rs2[:])

            # --- B2: knn d^2 ---
            nc.vector.tensor_sub(
                d25A[:], top8A[:, :, 1:1 + K],
                r2hA[:].unsqueeze(2).to_broadcast([128, RT, K]))
            nc.vector.tensor_scalar(d25A[:], d25A[:], -2.0, None, op0=ALU.mult)
            nc.vector.tensor_scalar_max(d25A[:], d25A[:], 1e-12)

            # --- B3: sqrt group (std, knn, dc) ---
            sd = sm.tile([128, RT], F32, tag="sd")
            nc.scalar.sqrt(sd[:], v1[:])
            nc.vector.tensor_scalar_add(
                featA[:, :, C + K + 1:C + K + 2].rearrange("p t o -> p (t o)"),
                sd[:], 1e-8)
            nc.scalar.sqrt(featA[:, :, C:C + K], d25A[:])
            nc.scalar.sqrt(dcA[:], dcA[:])

            # --- B4: softmax assign + intra (no max-subtraction) ---
            nc.vector.tensor_scalar_mul(
                zA[:].rearrange("p t c -> p (t c)"),
                dcA[:].rearrange("p t c -> p (t c)"), nIT[:])
            nc.scalar.activation(ezA[:], zA[:], ACTF.Exp)
            nc.vector.reduce_sum(seA[:], ezA[:], axis=AX.X)
            nc.vector.reciprocal(rseA[:], seA[:])
            nc.vector.tensor_mul(
                ezA[:], ezA[:],
                rseA[:].unsqueeze(2).to_broadcast([128, RT, C]))
            nc.vector.tensor_mul(
                featA[:, :, 0:C], ezA[:],
                cwB[:].unsqueeze(1).to_broadcast([128, RT, C]))
            dxa = wk.tile([128, RT, C], F32, tag="dxa")
            nc.vector.tensor_mul(dxa[:], dcA[:], featA[:, :, 0:C])
            nc.vector.tensor_reduce(intra_acc[:], dxa[:], axis=AX.XY,
                                    op=ALU.add)

            # --- entropy tail (Ln table last) ---
            nc.scalar.activation(lnseA[:], sexA[:], ACTF.Ln)
            nc.vector.tensor_sub(
                featA[:, :, C + K + 2:C + K + 3].rearrange("p t o -> p (t o)"),
                lnseA[:], t1[:])

            # --- B5: MLP ---
            # Layer 1 in fp16 (transpose is 1 matmul instead of an fp32
            # LOW/HIGH pair); layer 2 stays fp32. Loops are split per op
            # so the PE/ACT/DVE stages pipeline instead of ping-ponging.
            nc.vector.memset(featA[:, :, NF - 1:NF], 1.0)
            feat_h = big.tile([128, RT, NF], F16, tag="feat_h")
            nc.vector.tensor_copy(feat_h[:], featA[:])
            eye_h = cst.tile([128, 128], F16, tag="eye_h")
            nc.vector.tensor_copy(eye_h[:], eye[:])
            w1_h = cst.tile([NF, H], F16, tag="w1_h")
            nc.vector.tensor_copy(w1_h[:], w1[:])
            fThs = []
            for t in range(RT):
                fTp = pacc.tile([NF, 128], F16, tag="acc")
                nc.tensor.matmul(fTp[:], feat_h[:, t, :], eye_h[:],
                                 is_transpose=True, start=True, stop=True)
                fTh = w8.tile([NF, 128], F16, tag="fTh")
                nc.scalar.copy(fTh[:], fTp[:])
                fThs.append(fTh)
            hraTs = []
            for t in range(RT):
                hTp = pacc.tile([H, 128], F32, tag="acc")
                nc.tensor.matmul(hTp[:], w1_h[:], fThs[t][:],
                                 start=True, stop=True)
                hraT = w8.tile([H + 1, 128], F32, tag="hraT")
                nc.vector.tensor_scalar_max(hraT[0:H, :], hTp[:], 0.0)
                nc.vector.memset(hraT[H:H + 1, :], 1.0)
                hraTs.append(hraT)
            for t in range(RT):
                ep = pacc.tile([128, O], F32, tag="acc")
                nc.tensor.matmul(ep[:], hraTs[t][:], w2[:],
                                 start=True, stop=True)
                nc.scalar.copy(encA[:, t, :], ep[:])

            # ---------- batched output DMAs ----------
            nc.gpsimd.dma_start(
                knn_d.ap().rearrange("(t p) k -> p t k", p=128),
                featA[:, :, C:C + K])
            nc.gpsimd.dma_start(
                asn_d.ap().rearrange("(t p) c -> p t c", p=128),
                featA[:, :, 0:C])
            nc.gpsimd.dma_start(
                st_d.ap().rearrange("(t p) s -> p t s", p=128),
                featA[:, :, C + K:C + K + 3])
            nc.gpsimd.dma_start(
                enc_d.ap().rearrange("(t p) o -> p t o", p=128), encA[:])

            # ---------- intra partial reduce ----------
            ipp = pacc.tile([1, 1], F32, tag="acc")
            nc.tensor.matmul(ipp[:], intra_acc[:], ones_col[:],
                             start=True, stop=True)
            intra_sb = sm.tile([1, 1], F32, tag="s1c")
            nc.scalar.mul(intra_sb[:], ipp[:], 1.0 / (B * C))
            nc.gpsimd.dma_start(intra_d.ap(), intra_sb[:])

    nc.compile()
    return nc


def kernel(x, cluster_centers, temperature, cluster_weights, W1, b1, W2, b2,
           noise):
    del noise  # tie-break only; cannot change output values beyond ~1e-6
    x = np.asarray(x, dtype=np.float32)
    cc = np.asarray(cluster_centers, dtype=np.float32)
    temp = np.asarray(temperature, dtype=np.float32).reshape(1, 1)
    cw = np.asarray(cluster_weights, dtype=np.float32).reshape(1, C)
    W1 = np.asarray(W1, dtype=np.float32)
    b1 = np.asarray(b1, dtype=np.float32)
    W2 = np.asarray(W2, dtype=np.float32)
    b2 = np.asarray(b2, dtype=np.float32)

    if "nc" not in _CACHE:
        _CACHE["nc"] = _build()
    nc = _CACHE["nc"]

    xT = np.ascontiguousarray(x.T)                      # [256, 8192]
    f16 = np.float16
    xTh = np.ascontiguousarray(
        xT.reshape(2, 128, B).transpose(1, 0, 2)).astype(f16)
    ccT = np.ascontiguousarray(cc.T)
    W1a = np.concatenate([W1, b1.reshape(1, H)], axis=0)   # [17, 64]
    W2a = np.concatenate([W2, b2.reshape(1, O)], axis=0)
    eye = np.eye(128, dtype=np.float32)

    in_maps = []
    for c in range(NCORES):
        sl = slice(c * S, (c + 1) * S)
        xsT = np.ascontiguousarray(xT[:, sl])
        xsTh = np.ascontiguousarray(
            xsT.reshape(2, 128, S).transpose(1, 0, 2)).astype(f16)
        in_maps.append({
            "xTh": xTh,
            "xsTh": xsTh,
            "xs": np.ascontiguousarray(x[sl]),
            "ccTa": ccT,
            "cw": cw,
            "temp": temp,
            "W1a": W1a,
            "W2a": W2a,
            "eye": eye,
        })

    res = run_bass_kernel_spmd(nc, in_maps, core_ids=list(range(NCORES)))
    rs = res.results

    encoded = np.concatenate([r["enc"] for r in rs], axis=0)
    assign = np.concatenate([r["asn"] for r in rs], axis=0)
    knn = np.concatenate([r["knn"] for r in rs], axis=0)
    stats = np.concatenate([r["st"] for r in rs], axis=0)
    intra = np.float32(sum(float(r["intra"][0, 0]) for r in rs))
    inter = np.float32(rs[0]["inter"][0, 0])
    loss = np.float32(intra - 0.1 * inter)
    return encoded, assign, knn, stats, loss


# revision 29
# speedup vs baseline: 1.0442x; 1.0119x over previous
"""Trainium2 Bass kernel for AdvancedWeightPredictorNetwork (retrieval_knn).

Strategy (8 NeuronCores, data-parallel over rows of x):
  - Each core owns a 1024-row shard of x [8192, 256]; x^T is replicated
    (pre-laid-out in fp16 by the host as part of sharding).
  - Distance ranking: G' = x_shard @ x_all^T - r2_all/2 via fp16 matmuls
    into PSUM (k-chunks: two 128-feature halves + one row carrying
    -|x_j|^2/2, so ranking by G' equals ranking by -cdist^2).
    Per 2048-col PSUM chunk, vector.max (top-8, sorted desc) reduces
    the chunk straight from PSUM; a merge max over 4x8 candidates gives
    the global top-8 per row. Entry 0 is always the row itself; entries
    1..5 are the 5 nearest: knn = sqrt(r2_i - 2*s) with exact fp32
    r2_i (= sum x^2 from the local-stats pass). The tie-break noise
    matrix only permutes neighbors whose distances agree to ~1e-6 (far
    below tolerance), so it is not streamed.
  - Phase A is a pure dense matmul stream (PE clock-gate stays warm);
    phase B (cluster softmax / stats / MLP) is batched per op across
    row tiles to minimize ACT table reloads and DVE op count. exp() is
    applied without max-subtraction: inputs are bounded (|x| < 6,
    logits in [-25, 0]) so fp32 exp is safe and matches softmax math.
  - Scalar losses: per-core intra partial + inter computed on device;
    host sums the 8 partials.
"""

import numpy as np

import concourse.bacc as bacc
import concourse.tile as tile
import concourse.mybir as mybir
from concourse.bass_utils import run_bass_kernel_spmd

F32 = mybir.dt.float32
F16 = mybir.dt.float16
ALU = mybir.AluOpType
ACTF = mybir.ActivationFunctionType
AX = mybir.AxisListType

B = 8192        # total rows
NCORES = 8
S = B // NCORES  # rows per core (1024)
F = 256         # features
C = 8           # clusters
K = 5           # neighbors
H = 64          # hidden
O = 32          # output
RT = S // 128   # row tiles per core (8)
PCH = 2048      # psum chunk columns (4 banks)
NCH = B // PCH  # chunks per row tile (4)
NF = C + K + 4  # feat width incl. ones column (17)

_CACHE = {}


def _build():
    nc = bacc.Bacc("TRN2", target_bir_lowering=False, debug=False,
                   num_devices=NCORES)

    xTh_d = nc.dram_tensor("xTh", [128, 2, B], F16, kind="ExternalInput")
    xsTh_d = nc.dram_tensor("xsTh", [128, 2, S], F16, kind="ExternalInput")
    xs_d = nc.dram_tensor("xs", [S, F], F32, kind="ExternalInput")
    ccT_d = nc.dram_tensor("ccTa", [F, C], F32, kind="ExternalInput")
    cw_d = nc.dram_tensor("cw", [1, C], F32, kind="ExternalInput")
    temp_d = nc.dram_tensor("temp", [1, 1], F32, kind="ExternalInput")
    w1_d = nc.dram_tensor("W1a", [NF, H], F32, kind="ExternalInput")
    w2_d = nc.dram_tensor("W2a", [H + 1, O], F32, kind="ExternalInput")
    eye_d = nc.dram_tensor("eye", [128, 128], F32, kind="ExternalInput")

    enc_d = nc.dram_tensor("enc", [S, O], F32, kind="ExternalOutput")
    asn_d = nc.dram_tensor("asn", [S, C], F32, kind="ExternalOutput")
    knn_d = nc.dram_tensor("knn", [S, K], F32, kind="ExternalOutput")
    st_d = nc.dram_tensor("st", [S, 3], F32, kind="ExternalOutput")
    intra_d = nc.dram_tensor("intra", [1, 1], F32, kind="ExternalOutput")
    inter_d = nc.dram_tensor("inter", [1, 1], F32, kind="ExternalOutput")
    wu_d = nc.dram_tensor("wu", [1, 1], F32, kind="ExternalOutput")

    with tile.TileContext(nc) as tc:
        with (
            tc.tile_pool(name="big", bufs=1) as big,
            tc.tile_pool(name="sq", bufs=2) as sqp,
            tc.tile_pool(name="cst", bufs=1) as cst,
            tc.tile_pool(name="wk", bufs=2) as wk,
            tc.tile_pool(name="sm", bufs=4) as sm,
            tc.tile_pool(name="w8", bufs=8) as w8,
            tc.tile_pool(name="acc", bufs=2, space="PSUM") as pacc,
        ):
            # ---------- persistent tiles ----------
            xT_h = big.tile([128, 2, B], F16, tag="xT_h")
            r2row = big.tile([1, B], F16, tag="r2row")      # -r2/2
            xsT_h = big.tile([128, 2, S], F16, tag="xsT_h")
            intra_acc = big.tile([128, 1], F32, tag="intra_acc")
            top8A = big.tile([128, RT, 8], F32, tag="top8A")
            candA = big.tile([128, RT, NCH, 8], F32, tag="candA")
            featA = big.tile([128, RT, NF], F32, tag="featA")
            dcA = big.tile([128, RT, C], F32, tag="dcA")
            xrA = big.tile([128, RT, F], F32, tag="xrA")
            encA = big.tile([128, RT, O], F32, tag="encA")
            r2hA = big.tile([128, RT], F32, tag="r2hA")     # +r2/2 exact
            ssqA = big.tile([128, RT], F32, tag="ssqA")
            lsumA = big.tile([128, RT], F32, tag="lsumA")
            sexA = big.tile([128, RT], F32, tag="sexA")
            pxsA = big.tile([128, RT], F32, tag="pxsA")
            lnseA = big.tile([128, RT], F32, tag="lnseA")
            zA = big.tile([128, RT, C], F32, tag="zA")
            ezA = big.tile([128, RT, C], F32, tag="ezA")
            seA = big.tile([128, RT], F32, tag="seA")
            rseA = big.tile([128, RT], F32, tag="rseA")
            d25A = big.tile([128, RT, K], F32, tag="d25A")

            eye = cst.tile([128, 128], F32, tag="eye")
            ccT = cst.tile([128, 2, C], F32, tag="ccT")
            ccsq = cst.tile([128, 2, C], F32, tag="ccsq")
            ccT_h = cst.tile([128, 2, C], F16, tag="ccT_h")
            c2n_h = cst.tile([1, C], F16, tag="c2n_h")
            c2n = cst.tile([1, C], F32, tag="c2n")       # -c2/2
            c2p = cst.tile([1, C], F32, tag="c2p")       # +c2/2
            c2col = cst.tile([C, 1], F32, tag="c2col")   # +c2/2 column
            cwB = cst.tile([128, C], F32, tag="cwB")
            nIT = cst.tile([128, 1], F32, tag="nIT")     # -1/temp bcast
            w1 = cst.tile([NF, H], F32, tag="w1")
            w2 = cst.tile([H + 1, O], F32, tag="w2")
            cw_sb = cst.tile([1, C], F32, tag="cw_sb")
            t_sb = cst.tile([1, 1], F32, tag="t_sb")
            nrT = cst.tile([1, 1], F32, tag="nrT")
            ones_f16 = cst.tile([128, 1], F16, tag="ones_f16")
            one_f16r = cst.tile([1, 128], F16, tag="one_f16r")
            ones1 = cst.tile([1, 128], F32, tag="ones1")
            ones_col = cst.tile([128, 1], F32, tag="ones_col")

            # ---------- input DMAs (fast path first on sync queue) -------
            nc.sync.dma_start(xsT_h[:], xsTh_d.ap())
            for p in range(NCH):
                psl = slice(p * PCH, (p + 1) * PCH)
                nc.sync.dma_start(xT_h[:, :, psl], xTh_d.ap()[:, :, psl])
            # Secondary inputs share the same (serial) queue AFTER the xT
            # pieces so they don't steal HBM bandwidth from the matmul-
            # gating loads during the 8-core startup window.
            nc.sync.dma_start(
                xrA[:], xs_d.ap().rearrange("(t p) f -> p t f", p=128))
            for h in range(2):
                nc.sync.dma_start(ccT[:, h, :],
                                  ccT_d.ap()[h * 128:(h + 1) * 128, :])
            nc.sync.dma_start(cw_sb[:], cw_d.ap())
            nc.sync.dma_start(t_sb[:], temp_d.ap())
            nc.sync.dma_start(w1[:], w1_d.ap())
            nc.sync.dma_start(w2[:], w2_d.ap())
            nc.sync.dma_start(eye[:], eye_d.ap())
            nc.vector.memset(ones_f16[:], 1.0)
            nc.vector.memset(one_f16r[:], 1.0)
            nc.vector.memset(ones1[:], 1.0)
            nc.vector.memset(ones_col[:], 1.0)

            # PE warm-up during the input-DMA window: dense dummy matmuls
            # keep the HAM clock-gate busy so real matmuls start at 2.4 GHz.
            wu_s = cst.tile([128, 512], F16, tag="wu_s")
            nc.vector.memset(wu_s[:], 0.001)
            wup = pacc.tile([1, 512], F32, tag="acc")
            for i in range(40):
                nc.tensor.matmul(wup[:], ones_f16[:], wu_s[:],
                                 start=(i == 0), stop=(i == 39))
            wu_sb = sm.tile([1, 512], F32, tag="wu_sb")
            nc.scalar.copy(wu_sb[:], wup[:])
            wu_r = sm.tile([1, 1], F32, tag="wu_r")
            nc.vector.reduce_sum(wu_r[:], wu_sb[:], axis=AX.X)
            nc.gpsimd.dma_start(wu_d.ap(), wu_r[:])

            # ======== PHASE A: setup pieces then dense matmul stream =====
            def emit_chunk(t, c):
                tsl = slice(t * 128, (t + 1) * 128)
                acc = pacc.tile([128, PCH], F32, tag="acc")
                for k in range(2):
                    for n in range(PCH // 512):
                        csl = slice(c * PCH + n * 512, c * PCH + (n + 1) * 512)
                        nsl = slice(n * 512, (n + 1) * 512)
                        nc.tensor.matmul(acc[:, nsl], xsT_h[:, k, tsl],
                                         xT_h[:, k, csl],
                                         start=(k == 0), stop=False)
                for n in range(PCH // 512):
                    csl = slice(c * PCH + n * 512, c * PCH + (n + 1) * 512)
                    nsl = slice(n * 512, (n + 1) * 512)
                    nc.tensor.matmul(acc[:, nsl], one_f16r[:], r2row[:, csl],
                                     start=False, stop=True)
                nc.vector.max(candA[:, t, c, :], acc[:])

            def emit_piece(p):
                # piece p: squares + -r2/2 row for cols [p*2048,(p+1)*2048)
                psl = slice(p * PCH, (p + 1) * PCH)
                sq0 = sqp.tile([128, PCH], F16, tag="sq0")
                nc.vector.tensor_mul(sq0[:], xT_h[:, 0, psl], xT_h[:, 0, psl])
                sq1 = sqp.tile([128, PCH], F16, tag="sq1")
                nc.scalar.square(sq1[:], xT_h[:, 1, psl])
                for r in range(PCH // 512):
                    gsl = slice(p * PCH + r * 512, p * PCH + (r + 1) * 512)
                    lsl = slice(r * 512, (r + 1) * 512)
                    rp = pacc.tile([1, 512], F32, tag="acc")
                    nc.tensor.matmul(rp[:], ones_f16[:], sq0[:, lsl],
                                     start=True, stop=False)
                    nc.tensor.matmul(rp[:], ones_f16[:], sq1[:, lsl],
                                     start=False, stop=True)
                    nc.scalar.mul(r2row[0:1, gsl], rp[:], -0.5)

            def emit_cluster_setup():
                nc.vector.tensor_mul(ccsq[:], ccT[:], ccT[:])
                c2ps = pacc.tile([1, C], F32, tag="acc")
                for h in range(2):
                    nc.tensor.matmul(c2ps[:], ones_col[:], ccsq[:, h, :],
                                     start=(h == 0), stop=(h == 1))
                nc.scalar.mul(c2n[:], c2ps[:], -0.5)
                nc.vector.tensor_copy(ccT_h[:], ccT[:])
                nc.vector.tensor_copy(c2n_h[:], c2n[:])
                nc.scalar.mul(c2p[:], c2ps[:], 0.5)
                ccp = pacc.tile([C, 1], F32, tag="acc")
                nc.tensor.matmul(ccp[:], c2p[:], ones_col[0:1, 0:1],
                                 start=True, stop=True)
                nc.vector.tensor_copy(c2col[:], ccp[:])
                rT = sm.tile([1, 1], F32, tag="s1")
                nc.vector.reciprocal(rT[:], t_sb[:])
                nc.vector.tensor_scalar_mul(nrT[:], rT[:], -1.0)
                bp = pacc.tile([128, 1], F32, tag="acc")
                nc.tensor.matmul(bp[:], ones1[:], nrT[:],
                                 start=True, stop=True)
                nc.vector.tensor_copy(nIT[:], bp[:])
                cp = pacc.tile([128, C], F32, tag="acc")
                nc.tensor.matmul(cp[:], ones1[:], cw_sb[:],
                                 start=True, stop=True)
                nc.vector.tensor_copy(cwB[:], cp[:])

            def emit_dc():
                nc.vector.tensor_scalar_mul(r2hA[:], ssqA[:], 0.5)
                for t in range(RT):
                    tsl = slice(t * 128, (t + 1) * 128)
                    pc = pacc.tile([128, C], F32, tag="acc")
                    for k in range(2):
                        nc.tensor.matmul(pc[:], xsT_h[:, k, tsl],
                                         ccT_h[:, k, :],
                                         start=(k == 0), stop=False)
                    nc.tensor.matmul(pc[:], one_f16r[:], c2n_h[:],
                                     start=False, stop=True)
                    nc.vector.tensor_scalar(dcA[:, t, :], pc[:],
                                            r2hA[:, t:t + 1], -2.0,
                                            op0=ALU.subtract, op1=ALU.mult)
                nc.vector.tensor_scalar_max(dcA[:], dcA[:], 1e-12)

            exTs = []
            pxTs = []

            def emit_stats_phase(ph):
                # Local stats via ACT accumulators + gpsimd multiplies,
                # spread across c-rows so ACT never blocks piece setup.
                if ph == 0:
                    for t in range(RT):
                        j1 = wk.tile([128, F], F32, tag="j1")
                        nc.scalar.activation(j1[:], xrA[:, t, :], ACTF.Square,
                                             accum_out=ssqA[:, t:t + 1])
                elif ph == 1:
                    for t in range(RT):
                        exT = w8.tile([128, F], F32, tag="exT")
                        nc.scalar.activation(exT[:], xrA[:, t, :], ACTF.Exp,
                                             accum_out=sexA[:, t:t + 1])
                        exTs.append(exT)
                elif ph == 2:
                    for t in range(RT):
                        pxT = w8.tile([128, F], F32, tag="pxT")
                        nc.gpsimd.tensor_mul(pxT[:], exTs[t][:], xrA[:, t, :])
                        pxTs.append(pxT)
                    for t in range(RT):
                        j3 = wk.tile([128, F], F32, tag="j3")
                        nc.scalar.activation(j3[:], pxTs[t][:], ACTF.Copy,
                                             accum_out=pxsA[:, t:t + 1])
                else:
                    for t in range(RT):
                        j2 = wk.tile([128, F], F32, tag="j2")
                        nc.scalar.activation(j2[:], xrA[:, t, :], ACTF.Copy,
                                             accum_out=lsumA[:, t:t + 1])

            for c in range(NCH):
                emit_piece(c)
                for t in range(RT):
                    emit_chunk(t, c)
                    if c == 0 and t == 1:
                        emit_cluster_setup()
                    if c == 0 and t == 3:
                        for _ph in range(4):
                            emit_stats_phase(_ph)
                    if c == 2 and t == 6:
                        emit_dc()
            for t in range(RT):
                nc.vector.max(top8A[:, t, :],
                              candA[:, t, :, :].rearrange("p a b -> p (a b)"))
            # --- inter-cluster loss (identical on all cores) ---
            g8 = pacc.tile([C, C], F32, tag="acc")
            for h in range(2):
                nc.tensor.matmul(g8[:], ccT[:, h, :], ccT[:, h, :],
                                 start=(h == 0), stop=False)
            nc.tensor.matmul(g8[:], ones1[0:1, 0:C], c2n[:],
                             start=False, stop=True)
            d2cc = sm.tile([C, C], F32, tag="d2cc")
            nc.vector.tensor_scalar(d2cc[:], g8[:], c2col[:], -2.0,
                                    op0=ALU.subtract, op1=ALU.mult)
            nc.vector.tensor_scalar_max(d2cc[:], d2cc[:], 1e-12)
            ccd = sm.tile([C, C], F32, tag="ccd")
            nc.scalar.sqrt(ccd[:], d2cc[:])
            crs = sm.tile([C, 1], F32, tag="crs")
            nc.vector.reduce_sum(crs[:], ccd[:], axis=AX.X)
            ip8 = pacc.tile([1, 1], F32, tag="acc")
            nc.tensor.matmul(ip8[:], crs[:], ones_col[0:C, 0:1],
                             start=True, stop=True)
            inter_sb = sm.tile([1, 1], F32, tag="s1b")
            nc.scalar.mul(inter_sb[:], ip8[:], 1.0 / (C * (C - 1)))
            nc.gpsimd.dma_start(inter_d.ap(), inter_sb[:])


            # ================= PHASE B: post-processing ==================
            nc.vector.tensor_scalar_mul(
                featA[:, :, C + K:C + K + 1].rearrange("p t o -> p (t o)"),
                lsumA[:], 1.0 / F)
            lm2 = sm.tile([128, RT], F32, tag="lm2")
            nc.vector.tensor_mul(
                lm2[:],
                featA[:, :, C + K:C + K + 1].rearrange("p t o -> p (t o)"),
                featA[:, :, C + K:C + K + 1].rearrange("p t o -> p (t o)"))
            v1 = sm.tile([128, RT], F32, tag="v1")
            nc.vector.scalar_tensor_tensor(
                out=v1[:], in0=lm2[:], scalar=-float(F), in1=ssqA[:],
                op0=ALU.mult, op1=ALU.add)
            nc.vector.tensor_scalar_mul(v1[:], v1[:], 1.0 / (F - 1))
            rs2 = sm.tile([128, RT], F32, tag="rs2")
            nc.vector.reciprocal(rs2[:], sexA[:])
            t1 = sm.tile([128, RT], F32, tag="t1")
            nc.vector.tensor_mul(t1[:], pxsA[:], rs2[:])

            # --- B2: knn d^2 ---
            nc.vector.tensor_sub(
                d25A[:], top8A[:, :, 1:1 + K],
                r2hA[:].unsqueeze(2).to_broadcast([128, RT, K]))
            nc.vector.tensor_scalar(d25A[:], d25A[:], -2.0, None, op0=ALU.mult)
            nc.vector.tensor_scalar_max(d25A[:], d25A[:], 1e-12)

            # --- B3: sqrt group (std, knn, dc) ---
            sd = sm.tile([128, RT], F32, tag="sd")
            nc.scalar.sqrt(sd[:], v1[:])
            nc.vector.tensor_scalar_add(
                featA[:, :, C + K + 1:C + K + 2].rearrange("p t o -> p (t o)"),
                sd[:], 1e-8)
            nc.scalar.sqrt(featA[:, :, C:C + K], d25A[:])
            nc.scalar.sqrt(dcA[:], dcA[:])

            # --- B4: softmax assign + intra (no max-subtraction) ---
            nc.vector.tensor_scalar_mul(
                zA[:].rearrange("p t c -> p (t c)"),
                dcA[:].rearrange("p t c -> p (t c)"), nIT[:])
            nc.scalar.activation(ezA[:], zA[:], ACTF.Exp)
            nc.vector.reduce_sum(seA[:], ezA[:], axis=AX.X)
            nc.vector.reciprocal(rseA[:], seA[:])
            nc.vector.tensor_mul(
                ezA[:], ezA[:],
                rseA[:].unsqueeze(2).to_broadcast([128, RT, C]))
            nc.vector.tensor_mul(
                featA[:, :, 0:C], ezA[:],
                cwB[:].unsqueeze(1).to_broadcast([128, RT, C]))
            dxa = wk.tile([128, RT, C], F32, tag="dxa")
            nc.vector.tensor_mul(dxa[:], dcA[:], featA[:, :, 0:C])
            nc.vector.tensor_reduce(intra_acc[:], dxa[:], axis=AX.XY,
                                    op=ALU.add)

            # --- entropy tail (Ln table last) ---
            nc.scalar.activation(lnseA[:], sexA[:], ACTF.Ln)
            nc.vector.tensor_sub(
                featA[:, :, C + K + 2:C + K + 3].rearrange("p t o -> p (t o)"),
                lnseA[:], t1[:])

            # --- B5: MLP ---
            # Layer 1 in fp16 (transpose is 1 matmul instead of an fp32
            # LOW/HIGH pair); layer 2 stays fp32. Loops are split per op
            # so the PE/ACT/DVE stages pipeline instead of ping-ponging.
            nc.vector.memset(featA[:, :, NF - 1:NF], 1.0)
            feat_h = big.tile([128, RT, NF], F16, tag="feat_h")
            nc.vector.tensor_copy(feat_h[:], featA[:])
            eye_h = cst.tile([128, 128], F16, tag="eye_h")
            nc.vector.tensor_copy(eye_h[:], eye[:])
            w1_h = cst.tile([NF, H], F16, tag="w1_h")
            nc.vector.tensor_copy(w1_h[:], w1[:])
            fThs = []
            for t in range(RT):
                fTp = pacc.tile([NF, 128], F16, tag="acc")
                nc.tensor.matmul(fTp[:], feat_h[:, t, :], eye_h[:],
                                 is_transpose=True, start=True, stop=True)
                fTh = w8.tile([NF, 128], F16, tag="fTh")
                nc.scalar.copy(fTh[:], fTp[:])
                fThs.append(fTh)
            hraTs = []
            for t in range(RT):
                hTp = pacc.tile([H, 128], F32, tag="acc")
                nc.tensor.matmul(hTp[:], w1_h[:], fThs[t][:],
                                 start=True, stop=True)
                hraT = w8.tile([H + 1, 128], F32, tag="hraT")
                nc.vector.tensor_scalar_max(hraT[0:H, :], hTp[:], 0.0)
                nc.vector.memset(hraT[H:H + 1, :], 1.0)
                hraTs.append(hraT)
            for t in range(RT):
                ep = pacc.tile([128, O], F32, tag="acc")
                nc.tensor.matmul(ep[:], hraTs[t][:], w2[:],
                                 start=True, stop=True)
                nc.scalar.copy(encA[:, t, :], ep[:])

            # ---------- batched output DMAs ----------
            nc.gpsimd.dma_start(
                knn_d.ap().rearrange("(t p) k -> p t k", p=128),
                featA[:, :, C:C + K])
            nc.gpsimd.dma_start(
                asn_d.ap().rearrange("(t p) c -> p t c", p=128),
                featA[:, :, 0:C])
            nc.gpsimd.dma_start(
                st_d.ap().rearrange("(t p) s -> p t s", p=128),
                featA[:, :, C + K:C + K + 3])
            nc.gpsimd.dma_start(
                enc_d.ap().rearrange("(t p) o -> p t o", p=128), encA[:])

            # ---------- intra partial reduce ----------
            ipp = pacc.tile([1, 1], F32, tag="acc")
            nc.tensor.matmul(ipp[:], intra_acc[:], ones_col[:],
                             start=True, stop=True)
            intra_sb = sm.tile([1, 1], F32, tag="s1c")
            nc.scalar.mul(intra_sb[:], ipp[:], 1.0 / (B * C))
            nc.gpsimd.dma_start(intra_d.ap(), intra_sb[:])

    nc.compile()
    return nc


def kernel(x, cluster_centers, temperature, cluster_weights, W1, b1, W2, b2,
           noise):
    del noise  # tie-break only; cannot change output values beyond ~1e-6
    x = np.asarray(x, dtype=np.float32)
    cc = np.asarray(cluster_centers, dtype=np.float32)
    temp = np.asarray(temperature, dtype=np.float32).reshape(1, 1)
    cw = np.asarray(cluster_weights, dtype=np.float32).reshape(1, C)
    W1 = np.asarray(W1, dtype=np.float32)
    b1 = np.asarray(b1, dtype=np.float32)
    W2 = np.asarray(W2, dtype=np.float32)
    b2 = np.asarray(b2, dtype=np.float32)

    if "nc" not in _CACHE:
        _CACHE["nc"] = _build()
    nc = _CACHE["nc"]

    xT = np.ascontiguousarray(x.T)                      # [256, 8192]
    f16 = np.float16
    xTh = np.ascontiguousarray(
        xT.reshape(2, 128, B).transpose(1, 0, 2)).astype(f16)
    ccT = np.ascontiguousarray(cc.T)
    W1a = np.concatenate([W1, b1.reshape(1, H)], axis=0)   # [17, 64]
    W2a = np.concatenate([W2, b2.reshape(1, O)], axis=0)
    eye = np.eye(128, dtype=np.float32)

    in_maps = []
    for c in range(NCORES):
        sl = slice(c * S, (c + 1) * S)
        xsT = np.ascontiguousarray(xT[:, sl])
        xsTh = np.ascontiguousarray(
            xsT.reshape(2, 128, S).transpose(1, 0, 2)).astype(f16)
        in_maps.append({
            "xTh": xTh,
            "xsTh": xsTh,
            "xs": np.ascontiguousarray(x[sl]),
            "ccTa": ccT,
            "cw": cw,
            "temp": temp,
            "W1a": W1a,
            "W2a": W2a,
            "eye": eye,
        })

    res = run_bass_kernel_spmd(nc, in_maps, core_ids=list(range(NCORES)))
    rs = res.results

    encoded = np.concatenate([r["enc"] for r in rs], axis=0)
    assign = np.concatenate([r["asn"] for r in rs], axis=0)
    knn = np.concatenate([r["knn"] for r in rs], axis=0)
    stats = np.concatenate([r["st"] for r in rs], axis=0)
    intra = np.float32(sum(float(r["intra"][0, 0]) for r in rs))
    inter = np.float32(rs[0]["inter"][0, 0])
    loss = np.float32(intra - 0.1 * inter)
    return encoded, assign, knn, stats, loss
